# revision 5
# baseline (speedup 1.0000x reference)
"""Distributed Trainium2 kernel for the image-captioning model
(Linear+BN image embed -> 2-layer LSTM (T=64) -> H->V=32000 projection).

Sharding: the LSTM hidden state is sharded over the 4H gate dimension
(each of the 8 cores owns 128 h-positions of each layer); the per-step
full-h is re-assembled with one AllGather per pipeline stage (layer 1
runs one step behind layer 0, so both layers share a single exchange).
The fc projection is sharded over V (4000 rows/core) and interleaved
into the PE idle gaps of the recurrence. All matmuls run in bf16 with
fp32 PSUM accumulation.
"""
import numpy as np
import ml_dtypes

import concourse.bass as bass
import concourse.bacc as bacc
import concourse.mybir as mybir
from concourse.tile import TileContext
from concourse.tile_rust import add_dep_helper
from concourse.bass_utils import run_bass_kernel_spmd

BF16 = ml_dtypes.bfloat16
F32 = np.float32

V, H, E, B, T = 32000, 1024, 512, 32, 64
NC = 8
HS = H // NC          # 128 h-positions per core
VS = V // NC          # 4000 vocab rows per core
TOK = B * T           # 2048 tokens, col = t*32 + b
EPS = 1e-5
# gate slot order in psum: i, f, o, g  (PyTorch weight row blocks i,f,g,o)
GATE_BASE = [0, 1, 3, 2]

_nc_cache = None


def _gate_tiles(W, r, nk):
    """Per-core lhsT tile layout for a 4H-row weight: returns
    [128, 4*nk*128] with tile (j, k) at cols (j*nk+k)*128."""
    blocks = np.stack(
        [W[GATE_BASE[j] * H + r * HS : GATE_BASE[j] * H + r * HS + HS, :].T
         for j in range(4)]
    )  # (4, K, 128)
    K = blocks.shape[1]
    assert K == nk * 128
    return np.ascontiguousarray(
        blocks.reshape(4, nk, 128, 128).transpose(2, 0, 1, 3).reshape(128, 4 * nk * 128)
    ).astype(BF16)


def _prep(inputs):
    imgT = np.ascontiguousarray(inputs["image_feat"].T)  # (2048, 32)
    # rhs tiles [128, 16*32]
    imgT_s = np.ascontiguousarray(
        imgT.reshape(16, 128, B).transpose(1, 0, 2).reshape(128, 16 * B)
    ).astype(BF16)
    # lin lhsT tiles [128, 16k*4m*128]
    A = inputs["lin_W"].T  # (2048, 512)
    linWT = np.ascontiguousarray(
        A.reshape(16, 128, 4, 128).transpose(1, 0, 2, 3).reshape(128, 8192)
    ).astype(BF16)

    def col4(x):
        return np.ascontiguousarray(x.reshape(4, 128).T).astype(F32)

    bng = col4(inputs["bn_gamma"])
    bnb = col4(inputs["bn_beta"])

    caps = np.asarray(inputs["captions"])[:, : T - 1]  # (32, 63)
    cap_emb = inputs["emb"][caps]  # (32, 63, 512)
    capT = np.ascontiguousarray(cap_emb.transpose(2, 1, 0).reshape(E, (T - 1) * B)).astype(BF16)

    ident = np.eye(128, dtype=BF16)

    common = {
        "imgT": imgT_s, "linWT": linWT, "bng": bng, "bnb": bnb,
        "capT": capT, "ident": ident,
    }

    b0 = inputs["lstm_bih0"] + inputs["lstm_bhh0"]
    b1 = inputs["lstm_bih1"] + inputs["lstm_bhh1"]
    fcW = inputs["fc_W"]
    fcb_full = inputs["fc_b"]

    in_maps = []
    for r in range(NC):
        m = dict(common)
        m["w0i"] = _gate_tiles(inputs["lstm_Wih0"], r, 4)   # (128, 2048)
        m["w0h"] = _gate_tiles(inputs["lstm_Whh0"], r, 8)   # (128, 4096)
        m["w1i"] = _gate_tiles(inputs["lstm_Wih1"], r, 8)
        m["w1h"] = _gate_tiles(inputs["lstm_Whh1"], r, 8)
        m["b0"] = np.ascontiguousarray(
            np.stack([b0[GATE_BASE[j] * H + r * HS : GATE_BASE[j] * H + r * HS + HS]
                      for j in range(4)], axis=1)).astype(F32)  # (128, 4)
        m["b1"] = np.ascontiguousarray(
            np.stack([b1[GATE_BASE[j] * H + r * HS : GATE_BASE[j] * H + r * HS + HS]
                      for j in range(4)], axis=1)).astype(F32)
        F = np.zeros((4096, H), np.float32)
        F[:VS] = fcW[r * VS : (r + 1) * VS]
        m["fcw"] = np.ascontiguousarray(
            F.T.reshape(8, 128, 32, 128).transpose(1, 2, 0, 3).reshape(128, 32768)
        ).astype(BF16)
        fb = np.zeros((4096,), np.float32)
        fb[:VS] = fcb_full[r * VS : (r + 1) * VS]
        m["fcb"] = np.ascontiguousarray(fb.reshape(32, 128).T).astype(F32)
        in_maps.append(m)
    return in_maps


def _build():
    global _nc_cache
    if _nc_cache is not None:
        return _nc_cache
    bf = mybir.dt.bfloat16
    f32 = mybir.dt.float32
    nc = bacc.Bacc("TRN2", target_bir_lowering=False, debug=False)

    P = nc.declare_dram_parameter
    d_imgT = P("imgT", [128, 16 * B], bf, isOutput=False)
    d_linWT = P("linWT", [128, 8192], bf, isOutput=False)
    d_bng = P("bng", [128, 4], f32, isOutput=False)
    d_bnb = P("bnb", [128, 4], f32, isOutput=False)
    d_capT = P("capT", [E, (T - 1) * B], bf, isOutput=False)
    d_ident = P("ident", [128, 128], bf, isOutput=False)
    d_w0i = P("w0i", [128, 2048], bf, isOutput=False)
    d_w0h = P("w0h", [128, 4096], bf, isOutput=False)
    d_w1i = P("w1i", [128, 4096], bf, isOutput=False)
    d_w1h = P("w1h", [128, 4096], bf, isOutput=False)
    d_b0 = P("b0", [128, 4], f32, isOutput=False)
    d_b1 = P("b1", [128, 4], f32, isOutput=False)
    d_fcw = P("fcw", [128, 32768], bf, isOutput=False)
    d_fcb = P("fcb", [128, 32], f32, isOutput=False)
    d_out = P("out", [VS, TOK], f32, isOutput=True)

    S = nc.alloc_sbuf_tensor
    linWT_s = S("linWT_s", [128, 8192], bf)
    imgT_s = S("imgT_s", [128, 16 * B], bf)
    w0i_s = S("w0i_s", [128, 2048], bf)
    w0h_s = S("w0h_s", [128, 4096], bf)
    w1i_s = S("w1i_s", [128, 4096], bf)
    w1h_s = S("w1h_s", [128, 4096], bf)
    fcw_s = S("fcw_s", [128, 32768], bf)
    xsT_s = S("xsT_s", [128, 8192], bf)
    xp_s = S("xp_s", [128, 8192], bf)
    hs1_s = S("hs1_s", [128, 16384], bf)
    hbuf_s = [S(f"hbuf{i}", [128, NC * 64], bf) for i in range(4)]
    hsend_s = [S(f"hsend{i}", [128, 64], bf) for i in range(2)]
    c0_s = S("c0_s", [128, B], f32)
    c1_s = S("c1_s", [128, B], f32)
    sg0_s = [S(f"sg0_{i}", [128, 128], f32) for i in range(2)]
    sg1_s = [S(f"sg1_{i}", [128, 128], f32) for i in range(2)]
    fst_s = [S(f"fst{i}", [128, 512], f32) for i in range(2)]
    b0_s = S("b0_s", [128, 4], f32)
    b1_s = S("b1_s", [128, 4], f32)
    fcb_s = S("fcb_s", [128, 32], f32)
    bng_s = S("bng_s", [128, 4], f32)
    bnb_s = S("bnb_s", [128, 4], f32)
    ident_s = S("ident_s", [128, 128], bf)
    mu_s = S("mu_s", [128, 4], f32)
    e2_s = S("e2_s", [128, 4], f32)
    var_s = S("var_s", [128, 4], f32)
    sc_s = S("sc_s", [128, 4], f32)
    sh_s = S("sh_s", [128, 4], f32)
    tsq_s = S("tsq_s", [128, 128], f32)
    t1_s = S("t1_s", [128, B], f32)
    t2_s = S("t2_s", [128, B], f32)
    tc_s = S("tc_s", [128, B], f32)
    u1_s = S("u1_s", [128, B], f32)
    u2_s = S("u2_s", [128, B], f32)
    uc_s = S("uc_s", [128, B], f32)

    PS = nc.alloc_psum_tensor
    gb = [PS(f"gb{i}", [128, 512], f32) for i in range(4)]
    fbk = [PS(f"fb{i}", [128, 512], f32) for i in range(2)]

    cc_in = [nc.dram_tensor(f"cc_in{i}", [128, 64], bf) for i in range(2)]
    cc_out = [nc.dram_tensor(f"cc_out{i}", [NC * 128, 64], bf, addr_space="Shared")
              for i in range(2)]

    ACT = mybir.ActivationFunctionType
    AX = mybir.AxisListType

    with TileContext(nc) as tc:
        dma = nc.sync.dma_start
        # --- weight / const loads ---
        dma(out=linWT_s[:], in_=d_linWT[:])
        dma(out=imgT_s[:], in_=d_imgT[:])
        dma(out=w0i_s[:], in_=d_w0i[:])
        dma(out=w0h_s[:], in_=d_w0h[:])
        dma(out=w1i_s[:], in_=d_w1i[:])
        dma(out=w1h_s[:], in_=d_w1h[:])
        dma(out=fcw_s[:], in_=d_fcw[:])
        dma(out=b0_s[:], in_=d_b0[:])
        dma(out=b1_s[:], in_=d_b1[:])
        dma(out=fcb_s[:], in_=d_fcb[:])
        dma(out=bng_s[:], in_=d_bng[:])
        dma(out=bnb_s[:], in_=d_bnb[:])
        dma(out=ident_s[:], in_=d_ident[:])
        for k in range(4):
            dma(out=xsT_s[:, k * 2048 + B : (k + 1) * 2048],
                in_=d_capT[k * 128 : (k + 1) * 128, :])

        # --- image embed: x.T tiles -> gb[0][:, 0:128] ---
        for m in range(4):
            for k in range(16):
                nc.tensor.matmul(
                    gb[0][:, m * B : (m + 1) * B],
                    linWT_s[:, (k * 4 + m) * 128 : (k * 4 + m + 1) * 128],
                    imgT_s[:, k * B : (k + 1) * B],
                    start=(k == 0), stop=(k == 15),
                )
        # --- BN stats over batch (free dim) ---
        for m in range(4):
            nc.vector.reduce_sum(mu_s[:, m : m + 1], gb[0][:, m * B : (m + 1) * B], axis=AX.X)
        nc.scalar.activation(tsq_s[:], gb[0][:, 0:128], ACT.Square)
        for m in range(4):
            nc.vector.reduce_sum(e2_s[:, m : m + 1], tsq_s[:, m * B : (m + 1) * B], axis=AX.X)
        nc.scalar.mul(mu_s[:], mu_s[:], 1.0 / B)
        nc.scalar.mul(e2_s[:], e2_s[:], 1.0 / B)
        nc.vector.tensor_mul(var_s[:], mu_s[:], mu_s[:])
        nc.vector.tensor_sub(var_s[:], e2_s[:], var_s[:])
        nc.vector.tensor_scalar_add(var_s[:], var_s[:], EPS)
        nc.scalar.activation(var_s[:], var_s[:], ACT.Sqrt)
        nc.vector.reciprocal(var_s[:], var_s[:])
        nc.vector.tensor_mul(sc_s[:], bng_s[:], var_s[:])
        nc.vector.tensor_mul(sh_s[:], mu_s[:], sc_s[:])
        nc.vector.tensor_sub(sh_s[:], bnb_s[:], sh_s[:])
        for m in range(4):
            nc.scalar.activation(
                xsT_s[:, m * 2048 : m * 2048 + B], gb[0][:, m * B : (m + 1) * B],
                ACT.Identity, bias=sh_s[:, m : m + 1], scale=sc_s[:, m : m + 1],
            )

        # --- bulk pre0 = Wih0 @ xs (token-major, bias folded) -> xp ---
        pre_banks = [fbk[0], fbk[1], gb[0], gb[1]]
        for c in range(4):
            for j in range(4):
                pb = pre_banks[j]
                for k in range(4):
                    nc.tensor.matmul(
                        pb[:, 0:512],
                        w0i_s[:, (j * 4 + k) * 128 : (j * 4 + k + 1) * 128],
                        xsT_s[:, k * 2048 + c * 512 : k * 2048 + (c + 1) * 512],
                        start=(k == 0), stop=(k == 3),
                    )
            for j in range(4):
                pb = pre_banks[j]
                dst = xp_s[:].rearrange("p (t jj b) -> p t jj b", t=64, jj=4)[
                    :, c * 16 : (c + 1) * 16, j, :
                ]
                nc.scalar.activation(dst, pb[:, 0:512], ACT.Identity, bias=b0_s[:, j : j + 1])

        nc.vector.memset(c0_s[:], 0.0)
        nc.vector.memset(c1_s[:], 0.0)

        # --- FC work queue: (chunk, mtile) ready after hs1 holds its tokens ---
        fc_queue = [(c, m) for c in range(4) for m in range(32)]
        fc_idx = 0
        fc_count = 0

        def emit_fc(c, m):
            nonlocal fc_count
            pb = fbk[fc_count % 2]
            for k in range(8):
                nc.tensor.matmul(
                    pb[:, 0:512],
                    fcw_s[:, (m * 8 + k) * 128 : (m * 8 + k + 1) * 128],
                    hs1_s[:, k * 2048 + c * 512 : k * 2048 + (c + 1) * 512],
                    start=(k == 0), stop=(k == 7),
                )
            st = fst_s[fc_count % 2]
            nc.scalar.activation(st[:], pb[:, 0:512], ACT.Identity, bias=fcb_s[:, m : m + 1])
            rows = 128 if m < 31 else VS - 31 * 128
            dma(out=d_out[m * 128 : m * 128 + rows, c * 512 : (c + 1) * 512],
                in_=st[0:rows, :])
            fc_count += 1

        # --- 65 pipeline stages ---
        for t in range(T + 1):
            hb = hbuf_s[t % 4]
            nhb = hbuf_s[(t + 1) % 4]
            snd = hsend_s[t % 2]

            if t < T:
                # layer 0 step t (uses gathered h0_{t-1} in hb)
                g = gb[t % 2]
                if t > 0:
                    for j in range(4):
                        for k in range(8):
                            nc.tensor.matmul(
                                g[:, j * B : (j + 1) * B],
                                w0h_s[:, (j * 8 + k) * 128 : (j * 8 + k + 1) * 128],
                                hb[:, k * 64 : k * 64 + B],
                                start=(j == 0 and k == 0), stop=False,
                            )
                    nc.tensor.matmul(
                        g[:, 0:128], ident_s[:], xp_s[:, t * 128 : (t + 1) * 128],
                        start=False, stop=True,
                    )
                else:
                    nc.tensor.matmul(
                        g[:, 0:128], ident_s[:], xp_s[:, 0:128], start=True, stop=True
                    )
                sg = sg0_s[t % 2]
                nc.scalar.activation(sg[:, 0:96], g[:, 0:96], ACT.Sigmoid)
                nc.scalar.activation(sg[:, 96:128], g[:, 96:128], ACT.Tanh)
                nc.vector.tensor_mul(t1_s[:], sg[:, 32:64], c0_s[:])
                nc.vector.tensor_mul(t2_s[:], sg[:, 0:32], sg[:, 96:128])
                nc.vector.tensor_add(c0_s[:], t1_s[:], t2_s[:])
                nc.scalar.activation(tc_s[:], c0_s[:], ACT.Tanh)
                nc.vector.tensor_mul(snd[:, 0:B], sg[:, 64:96], tc_s[:])

            if t >= 1:
                # layer 1 step t-1 (input h0_{t-1} and state h1_{t-2}, both in hb)
                g = gb[2 + (t % 2)]
                for j in range(4):
                    for k in range(8):
                        nc.tensor.matmul(
                            g[:, j * B : (j + 1) * B],
                            w1i_s[:, (j * 8 + k) * 128 : (j * 8 + k + 1) * 128],
                            hb[:, k * 64 : k * 64 + B],
                            start=(j == 0 and k == 0),
                            stop=(t == 1 and j == 3 and k == 7),
                        )
                if t >= 2:
                    for j in range(4):
                        for k in range(8):
                            nc.tensor.matmul(
                                g[:, j * B : (j + 1) * B],
                                w1h_s[:, (j * 8 + k) * 128 : (j * 8 + k + 1) * 128],
                                hb[:, k * 64 + B : (k + 1) * 64],
                                start=False,
                                stop=(j == 3 and k == 7),
                            )
                sg = sg1_s[t % 2]
                nc.scalar.activation(sg[:, 0:32], g[:, 0:32], ACT.Sigmoid, bias=b1_s[:, 0:1])
                nc.scalar.activation(sg[:, 32:64], g[:, 32:64], ACT.Sigmoid, bias=b1_s[:, 1:2])
                nc.scalar.activation(sg[:, 64:96], g[:, 64:96], ACT.Sigmoid, bias=b1_s[:, 2:3])
                nc.scalar.activation(sg[:, 96:128], g[:, 96:128], ACT.Tanh, bias=b1_s[:, 3:4])
                nc.vector.tensor_mul(u1_s[:], sg[:, 32:64], c1_s[:])
                nc.vector.tensor_mul(u2_s[:], sg[:, 0:32], sg[:, 96:128])
                nc.vector.tensor_add(c1_s[:], u1_s[:], u2_s[:])
                nc.scalar.activation(uc_s[:], c1_s[:], ACT.Tanh)
                nc.vector.tensor_mul(snd[:, B:64], sg[:, 64:96], uc_s[:])

            if t >= 2:
                # harvest h1_{t-2} from hb into hs1 (token-major)
                src = hb[:].rearrange("p (s c) -> p s c", s=NC)[:, :, B:64]
                dst = hs1_s[:].rearrange("p (s tok) -> p s tok", s=NC)[
                    :, :, (t - 2) * B : (t - 1) * B
                ]
                nc.vector.tensor_copy(dst, src)

            # exchange: AllGather (h0_t, h1_{t-1})
            dma(out=cc_in[t % 2][:], in_=snd[:])
            nc.gpsimd.collective_compute(
                "AllGather",
                mybir.AluOpType.bypass,
                replica_groups=[list(range(NC))],
                ins=[cc_in[t % 2][:]],
                outs=[cc_out[t % 2][:]],
            )
            dma(
                out=nhb[:].rearrange("p (s c) -> p s c", s=NC),
                in_=cc_out[t % 2][:].rearrange("(s p) c -> p s c", s=NC),
            )

            # interleave up to 2 ready fc tiles per stage
            emitted = 0
            while fc_idx < len(fc_queue) and emitted < 2:
                c, m = fc_queue[fc_idx]
                if t < 16 * c + 18:
                    break
                emit_fc(c, m)
                fc_idx += 1
                emitted += 1

        # final harvest: h1_{T-1} was gathered by the stage-T AllGather
        fhb = hbuf_s[(T + 1) % 4]
        src = fhb[:].rearrange("p (s c) -> p s c", s=NC)[:, :, B:64]
        dst = hs1_s[:].rearrange("p (s tok) -> p s tok", s=NC)[
            :, :, (T - 1) * B : T * B
        ]
        nc.vector.tensor_copy(dst, src)

        while fc_idx < len(fc_queue):
            c, m = fc_queue[fc_idx]
            emit_fc(c, m)
            fc_idx += 1

    nc.finalize()
    _nc_cache = nc
    return nc


def kernel(**inputs):
    inputs = {k: np.asarray(v) for k, v in inputs.items()}
    in_maps = _prep(inputs)
    nc = _build()
    res = run_bass_kernel_spmd(nc, in_maps, core_ids=list(range(NC)))
    big = np.concatenate([np.asarray(res.results[r]["out"]) for r in range(NC)], axis=0)
    return np.ascontiguousarray(
        big.reshape(V, T, B).transpose(2, 1, 0)
    ).astype(np.float32)
